# revision 8
# baseline (speedup 1.0000x reference)
"""Trainium2 kernel v4 for the CrosscoderModule (encode -> top-k -> sparse decode).

Contract: kernel(**inputs) takes the FULL unsharded inputs
  x      (4096, 2, 1024) f32
  W_enc  (2, 1024, 32768) f32
  W_dec  (32768, 2, 1024) f32
  b_enc  (32768,) f32
  b_dec  (2, 1024) f32
and returns x_hat (4096, 2, 1024) f32.

Data-parallel over batch on 8 NeuronCores (512 rows each), no collectives.

v4: same algorithm as v3 (exact f32 encode, per-block top-8 candidates via
DVE max8/max_index, top-32 among 512 candidates, f16 W_dec row gathers), but
the three hot loops (64 d_sae blocks, 32-way index extraction, 32-way decode
accumulate) are hardware For_i loops instead of fully unrolled Python loops,
and decode accumulates on the DVE (scalar_tensor_tensor fused mul-add) rather
than diag-matmuls on the PE.  The body repeat count is a runtime input
(`niter`) driving an outer hardware loop, so timing harnesses can amplify
marginal device time against one compiled executable.  Static instruction
count per core drops from ~7k (v3) to ~780 while device-side work stays
~5.8 ms/core/iteration (measured via niter=1 vs 1025 wall delta).
"""

import sys

if "/opt/trn_rl_repo" not in sys.path:
    sys.path.insert(0, "/opt/trn_rl_repo")

import numpy as np

import concourse.bacc as bacc
import concourse.bass as bass
import concourse.mybir as mybir
from concourse.bass import ds
from concourse.bass_utils import run_bass_kernel_spmd
from concourse.tile import TileContext
from concourse.vector_clock import ScopedClock

# Problem dims
B, NPOS, DM, S, K = 4096, 2, 1024, 32768, 32
D = NPOS * DM              # 2048 contraction / output width
NCORES = 8
BC = B // NCORES           # 512 batch rows per core
P = 128                    # partitions

# Tiling
NBLK = S // 512            # 64 d_sae blocks of 512
KCH = D // P               # 16 contraction chunks of 128
BT = BC // P               # 4 batch tiles of 128 per core
NEG = -3.0e38

F32 = mybir.dt.float32
F16 = mybir.dt.float16
U32 = mybir.dt.uint32
F32R = mybir.dt.float32r

# Encode matmuls as float32r (1 PE cycle/row vs 4 for plain f32).  Off by
# default: fp32r hardware numerics are reduced-precision and must be
# validated against the top-k selection before enabling.
ENC_F32R = False


class SafeTileContext(TileContext):
    # Walrus rejects >2 sem waits on one SP CTRL instruction; spread the tail
    # drain's global-clock waits across single-wait nops first.
    def _drain_and_barrier(self, tick_clock, wait_clock):
        probe = self.nc.sync.nop()
        wait_clock.add_sem_waits(
            probe.ins, ScopedClock({None: tick_clock.global_clock})
        )
        waits = list(probe.ins.sync_info.on_wait or []) if probe.ins.sync_info else []
        if len(waits) > 1:
            probe.ins.sync_info = mybir.SyncInfo(on_wait=[waits[0]], on_update=[])
            for w in waits[1:]:
                n2 = self.nc.sync.nop()
                n2.ins.sync_info = mybir.SyncInfo(on_wait=[w], on_update=[])
        self.nc.sync.drain()
        self.nc.all_engine_barrier()
        assert self.sems is not None
        popped = self.nc._tile_sem_poison_stack.pop()
        assert popped is self._sem_poison
        self.nc.clear_and_free_semaphores(list(self.sems.allocated().values()))
        self.nc.all_engine_barrier()


def build_nc(nblk=NBLK, kch=KCH, bt_n=BT, n_iter=1, has_benc=False, unroll=2,
             enc_f32r=None):
    """Build the per-core program.  nblk/kch/bt_n shrink the problem for
    structural tests.  The body repeat count is a RUNTIME input (`niter`,
    default 1) driving an outer hardware loop; the build-time n_iter arg is
    accepted for compatibility but ignored."""
    cand = nblk * 8
    assert nblk % unroll == 0
    if enc_f32r is None:
        enc_f32r = ENC_F32R

    nc = bacc.Bacc("TRN2")
    FENC = F32R if enc_f32r else F32
    xt_d = nc.declare_dram_parameter("xt", [bt_n, kch, P, P], FENC, isOutput=False)
    w_d = nc.declare_dram_parameter("w", [nblk, kch, P, 512], FENC, isOutput=False)
    wd_d = nc.declare_dram_parameter("wdec", [S, D], F16, isOutput=False)
    if has_benc:
        be_d = nc.declare_dram_parameter("benc", [nblk, 512], F32, isOutput=False)
    bd_d = nc.declare_dram_parameter("bdec", [P, D], F32, isOutput=False)
    out_d = nc.declare_dram_parameter("out", [bt_n, P, D], F32, isOutput=True)
    tv_d = nc.declare_dram_parameter("top_vals", [bt_n, P, K], F32, isOutput=True)
    ti_d = nc.declare_dram_parameter("top_idx", [bt_n, P, K], U32, isOutput=True)
    nit_d = nc.declare_dram_parameter("niter", [1, 1], U32, isOutput=False)

    with SafeTileContext(nc) as tc:
        with (
            tc.tile_pool(name="const", bufs=1) as cpool,
            tc.tile_pool(name="x", bufs=1) as xpool,
            tc.tile_pool(name="w", bufs=1) as wpool,
            tc.tile_pool(name="cand", bufs=1) as candpool,
            tc.tile_pool(name="sel", bufs=1) as selpool,
            tc.tile_pool(name="dec", bufs=1) as decpool,
            tc.tile_pool(name="acc", bufs=1) as accpool,
            tc.tile_pool(name="psum", bufs=1, space="PSUM") as pspool,
        ):
            # constants
            basef = cpool.tile([P, cand], F32)     # candidate slot -> block base
            base_u = cpool.tile([P, cand], U32)
            nc.gpsimd.iota(base_u[:], pattern=[[512, nblk], [0, 8]], channel_multiplier=0)
            nc.vector.tensor_copy(basef[:], base_u[:])
            iota_cf = cpool.tile([P, cand], F32)   # 0..cand-1 per row
            iota_cu = cpool.tile([P, cand], U32)
            nc.gpsimd.iota(iota_cu[:], pattern=[[1, cand]], channel_multiplier=0)
            nc.vector.tensor_copy(iota_cf[:], iota_cu[:])
            bdec_t = cpool.tile([P, D], F32)
            nc.sync.dma_start(out=bdec_t[:], in_=bd_d[:])
            if has_benc:
                ones_t = cpool.tile([1, P], F32)
                nc.vector.memset(ones_t[:], 1.0)
            nit_t = cpool.tile([1, 1], U32)
            nc.sync.dma_start(out=nit_t[:], in_=nit_d[:])
            rt_n = nc.values_load(nit_t[:1, :1], min_val=1, max_val=1 << 20,
                                  skip_runtime_bounds_check=True)

            # outer repeat loop: count supplied at RUNTIME via the `niter`
            # input, so timing runs reuse one compiled executable
            with tc.For_i(0, rt_n, 1):
                # stationary x tiles, all kch chunks resident, all bt
                xt = {}
                for bt in range(bt_n):
                    t = xpool.tile([P, kch * P], FENC, tag=f"x{bt}", name=f"xt{bt}")
                    nc.sync.dma_start(
                        out=t[:].rearrange("p (k b) -> p k b", k=kch),
                        in_=xt_d[bt].rearrange("k p b -> p k b"),
                    )
                    xt[bt] = t
                cv_all = candpool.tile([P, bt_n * cand], F32, tag="cva", name="cv_all")
                cl_all = candpool.tile([P, bt_n * cand], U32, tag="cla", name="cl_all")
                cand_v = {bt: cv_all[:, bt * cand:(bt + 1) * cand] for bt in range(bt_n)}
                cand_loc = {bt: cl_all[:, bt * cand:(bt + 1) * cand] for bt in range(bt_n)}

                # ---- encode: HW loop over d_sae blocks, unrolled by `unroll`
                with tc.For_i(0, nblk, unroll) as iv:
                    for u in range(unroll):
                        nsym = iv + u
                        wt = wpool.tile([P, kch * 512], FENC, tag=f"w{u}", name=f"wt{u}")
                        nc.sync.dma_start(
                            out=wt[:].rearrange("p (k f) -> p k f", k=kch),
                            in_=w_d[nsym].rearrange("k p f -> p k f"),
                        )
                        if has_benc:
                            bench_t = wpool.tile([1, 512], F32, tag=f"bh{u}", name=f"bench{u}")
                            nc.sync.dma_start(out=bench_t[:], in_=be_d[ds(nsym, 1)])
                        acc = {}
                        for bt in range(bt_n):
                            acc[bt] = pspool.tile([P, 512], F32, tag=f"a{u}_{bt}",
                                                  name=f"a{u}_{bt}")
                        for k in range(kch):
                            wt_ap = wt[:, k * 512:(k + 1) * 512]
                            for bt in range(bt_n):
                                last = (k == kch - 1) and not has_benc
                                nc.tensor.matmul(
                                    acc[bt][:], xt[bt][:, k * P:(k + 1) * P], wt_ap,
                                    start=(k == 0), stop=last)
                        mv = wpool.tile([P, bt_n * 8], F32, tag=f"mv{u}", name=f"mv{u}")
                        ml = wpool.tile([P, bt_n * 8], U32, tag=f"ml{u}", name=f"ml{u}")
                        for bt in range(bt_n):
                            if has_benc:
                                nc.tensor.matmul(
                                    acc[bt][:], ones_t[:1, :], bench_t[:1, :],
                                    start=False, stop=True)
                            nc.vector.max(mv[:, bt * 8:(bt + 1) * 8], acc[bt][:])
                            nc.vector.max_index(
                                ml[:, bt * 8:(bt + 1) * 8],
                                mv[:, bt * 8:(bt + 1) * 8], acc[bt][:])
                        # scatter this block's 8 candidates into every bt's
                        # slot via one 3D SBUF->SBUF DMA (dynamic dest offset)
                        col = ds(nsym * 8, 8)
                        cv3 = cv_all[:].rearrange("p (b c) -> p b c", b=bt_n)
                        cl3 = cl_all[:].rearrange("p (b c) -> p b c", b=bt_n)
                        mv3 = mv[:].rearrange("p (b c) -> p b c", b=bt_n)
                        ml3 = ml[:].rearrange("p (b c) -> p b c", b=bt_n)
                        nc.sync.dma_start(out=cv3[:, :, col], in_=mv3)
                        nc.sync.dma_start(out=cl3[:, :, col], in_=ml3)

                # ---- selection: top-32 among the 8*nblk candidates
                candif = {}
                tv = {}
                slotsf = {}
                tif = {}
                ti = {}
                for bt in range(bt_n):
                    candif[bt] = selpool.tile([P, cand], F32, tag=f"cif{bt}", name=f"cif{bt}")
                    nc.vector.tensor_copy(candif[bt][:], cand_loc[bt][:])
                    nc.vector.tensor_add(candif[bt][:], candif[bt][:], basef[:])
                    nc.vector.tensor_scalar_add(candif[bt][:], candif[bt][:], 1.0)

                    tv[bt] = selpool.tile([P, K], F32, tag=f"tv{bt}", name=f"tv{bt}")
                    slots = selpool.tile([P, K], U32, tag=f"sl{bt}", name=f"sl{bt}")
                    for r in range(K // 8):
                        nc.vector.max(tv[bt][:, r * 8:(r + 1) * 8], cand_v[bt][:])
                        nc.vector.max_index(
                            slots[:, r * 8:(r + 1) * 8],
                            tv[bt][:, r * 8:(r + 1) * 8], cand_v[bt][:])
                        nc.vector.match_replace(
                            cand_v[bt][:], tv[bt][:, r * 8:(r + 1) * 8],
                            cand_v[bt][:], NEG)
                    slotsf[bt] = selpool.tile([P, K], F32, tag=f"sf{bt}", name=f"sf{bt}")
                    nc.vector.tensor_copy(slotsf[bt][:], slots[:])
                    tif[bt] = selpool.tile([P, K], F32, tag=f"tif{bt}", name=f"tif{bt}")

                # slot -> global index extraction, HW loop over the 32 slots
                eq = selpool.tile([P, cand], F32, tag="eq")
                prod = selpool.tile([P, cand], F32, tag="prod")
                red = selpool.tile([P, 1], F32, tag="red", name="red")
                with tc.For_i(0, K, 1) as jv:
                    for bt in range(bt_n):
                        nc.vector.tensor_scalar(
                            eq[:], iota_cf[:], slotsf[bt][:, ds(jv, 1)], None,
                            op0=mybir.AluOpType.is_equal)
                        nc.vector.tensor_mul(prod[:], eq[:], candif[bt][:])
                        nc.vector.reduce_max(
                            red[:], prod[:], axis=mybir.AxisListType.X)
                        nc.vector.tensor_scalar_add(
                            tif[bt][:, ds(jv, 1)], red[:], -1.0)

                for bt in range(bt_n):
                    ti[bt] = selpool.tile([P, K], U32, tag=f"ti{bt}", name=f"ti{bt}")
                    nc.vector.tensor_copy(ti[bt][:], tif[bt][:])
                    nc.vector.tensor_scalar_max(tv[bt][:], tv[bt][:], 0.0)
                    nc.sync.dma_start(out=tv_d[bt], in_=tv[bt][:])
                    nc.sync.dma_start(out=ti_d[bt], in_=ti[bt][:])

                # ---- decode: acc[bt] = b_dec + sum_j tv_j * Wdec16[idx_j]
                acc_t = {}
                for bt in range(bt_n):
                    acc_t[bt] = accpool.tile([P, D], F32, tag=f"acc{bt}", name=f"acc{bt}")
                    nc.vector.tensor_copy(acc_t[bt][:], bdec_t[:])
                with tc.For_i(0, K, 1) as jv2:
                    for bt in range(bt_n):
                        off = decpool.tile([P, 1], U32, tag=f"off{bt}", name=f"off{bt}")
                        nc.vector.tensor_copy(off[:], ti[bt][:, ds(jv2, 1)])
                        g = decpool.tile([P, D], F16, tag=f"g{bt}", name=f"g{bt}")
                        nc.gpsimd.indirect_dma_start(
                            out=g[:],
                            out_offset=None, in_=wd_d[:],
                            in_offset=bass.IndirectOffsetOnAxis(
                                ap=off[:, :1], axis=0))
                        nc.vector.scalar_tensor_tensor(
                            acc_t[bt][:], g[:], tv[bt][:, ds(jv2, 1)], acc_t[bt][:],
                            op0=mybir.AluOpType.mult,
                            op1=mybir.AluOpType.add)
                for bt in range(bt_n):
                    nc.sync.dma_start(out=out_d[bt], in_=acc_t[bt][:])
    nc.finalize()
    return nc


def prepare_inputs(x, W_enc, W_dec, b_enc, b_dec, nblk=NBLK, kch=KCH, bt_n=BT):
    """Host-side sharding + layout prep.  Returns per-core in_maps."""
    x = np.asarray(x, dtype=np.float32)
    W_enc = np.asarray(W_enc, dtype=np.float32)
    W_dec = np.asarray(W_dec, dtype=np.float32)
    b_enc = np.asarray(b_enc, dtype=np.float32)
    b_dec = np.asarray(b_dec, dtype=np.float32)

    dctr = kch * P
    sblk = nblk * 512
    W = W_enc.reshape(D, S)[:dctr, :sblk]
    # (dctr, sblk) -> (nblk, kch, 128, 512)
    w_t = np.ascontiguousarray(
        W.reshape(kch, P, nblk, 512).transpose(2, 0, 1, 3))
    wd = np.ascontiguousarray(W_dec.reshape(S, D).astype(np.float16))
    has_benc = bool(np.any(b_enc[:sblk]))
    bd = np.ascontiguousarray(np.broadcast_to(b_dec.reshape(1, D), (P, D)))

    in_maps = []
    for c in range(NCORES):
        xs = x[c * BC:(c + 1) * BC].reshape(BC, D)[: bt_n * P, :dctr]
        xT = np.ascontiguousarray(xs.T)  # (dctr, bt_n*128)
        xt = np.ascontiguousarray(
            xT.reshape(kch, P, bt_n, P).transpose(2, 0, 1, 3))
        m = {"xt": xt, "w": w_t, "wdec": wd, "bdec": bd,
             "niter": np.array([[1]], dtype=np.uint32)}
        if has_benc:
            m["benc"] = b_enc[:sblk].reshape(nblk, 512)
        in_maps.append(m)
    return in_maps


_NC_CACHE = {}


def kernel(x, W_enc, W_dec, b_enc, b_dec):
    in_maps = prepare_inputs(x, W_enc, W_dec, b_enc, b_dec)
    has_benc = "benc" in in_maps[0]
    key = (NBLK, KCH, BT, has_benc)
    if key not in _NC_CACHE:
        _NC_CACHE[key] = build_nc(has_benc=has_benc)
    nc = _NC_CACHE[key]
    res = run_bass_kernel_spmd(nc, in_maps, list(range(NCORES))).results
    out = np.concatenate([r["out"].reshape(BC, D) for r in res], axis=0)
    return out.reshape(B, NPOS, DM).astype(np.float32)


if __name__ == "__main__":
    rng = np.random.default_rng(0)
    ins = {
        "x": rng.standard_normal((B, NPOS, DM)).astype(np.float32),
        "W_enc": (rng.standard_normal((NPOS, DM, S)) / 32).astype(np.float32),
        "W_dec": (rng.standard_normal((S, NPOS, DM)) / 181).astype(np.float32),
        "b_enc": np.zeros(S, np.float32),
        "b_dec": np.zeros((NPOS, DM), np.float32),
    }
    y = kernel(**ins)
    print(y.shape, y.dtype)
